# revision 1
# baseline (speedup 1.0000x reference)
"""Trainium2 Bass kernel for nn_Attention_18786186952997.

Dense causal-attention transformer block with ternarized (BitNet-style)
weights and RoPE:

    wq = ternarize(w_qkv); wp = ternarize(w_proj)
    qkv = x @ wq.T ; q,k,v split ; RoPE(q,k) ; causal SDPA ; y @ wp.T

Sharding: 8 cores = 2 batches x 4 head-groups (4 heads each).  Each core
computes its batch's qkv projections for its 4 heads, runs causal
flash-style attention fully on-chip, and produces a partial (transposed)
projection output; the host sums the 4 partials per batch.

Device compute layout is channel-major ("transposed"): q.T/k.T are
produced as [head_dim, tokens] so RoPE and QK^T need no on-chip
transposes; exp(scores.T) is exactly the stationary layout that A@V
needs; softmax denominators come free from ones-columns appended to V.
Ternary weights are passed as exact bf16 sign matrices; the abs-mean
scales are folded into the exp() scale and the final output scale.
"""

import os
import sys
import types

import numpy as np

sys.path.insert(0, "/opt/trn_rl_repo")

import ml_dtypes  # noqa: E402

BF16 = ml_dtypes.bfloat16

B, T, C, H, D = 2, 2048, 1024, 16, 64
N_CORES = 8
HEADS_PER_CORE = 4
P = 128
QT = 512            # q tile (moving free dim)
NQT = T // QT       # 4
NKC = T // P        # 16 k chunks
NCC = C // P        # 8 contraction chunks

_CACHE = {}


def _install_ntff_hook():
    """bass_utils' trace=True path needs antenv.axon_hooks, absent in this
    image; synthesize it around the boot module's ctypes hook."""
    if "antenv.axon_hooks" in sys.modules:
        return
    try:
        import antenv  # noqa: F401
        from trn_agent_boot.trn_boot import _ntff_profile_via_ctypes
    except Exception:
        return
    mod = types.ModuleType("antenv.axon_hooks")
    holder = {}
    mod.set_axon_ntff_profile_hook = lambda h: holder.__setitem__("h", h)
    mod.get_axon_ntff_profile_hook = lambda: holder.get("h")
    sys.modules["antenv.axon_hooks"] = mod
    sys.modules["antenv"].axon_hooks = mod
    try:
        hook = _ntff_profile_via_ctypes("/opt/axon/libaxon_pjrt.so")
        mod.set_axon_ntff_profile_hook(hook)
    except Exception:
        pass


def _ternarize_host(w):
    """Sign matrix and abs-mean scale, bit-matching the jax reference."""
    try:
        import jax.numpy as jnp

        wj = jnp.asarray(w)
        am = jnp.maximum(jnp.abs(wj).mean(), 1e-5)
        thr = 0.7 * am
        s = jnp.where(wj > thr, 1.0, jnp.where(wj < -thr, -1.0, 0.0))
        return np.asarray(s, dtype=np.float32), np.float32(am)
    except Exception:
        am = np.float32(max(np.abs(w).astype(np.float32).mean(dtype=np.float32), 1e-5))
        thr = np.float32(0.7) * am
        s = np.where(w > thr, 1.0, np.where(w < -thr, -1.0, 0.0)).astype(np.float32)
        return s, am


def _build_program():
    import concourse.bass as bass  # noqa: F401
    import concourse.mybir as mybir
    import concourse.tile as tile
    from concourse import bacc

    F32 = mybir.dt.float32
    BF = mybir.dt.bfloat16
    AF = mybir.ActivationFunctionType

    nc = bacc.Bacc("TRN2", target_bir_lowering=False, debug=False,
                   num_devices=N_CORES)

    xt = nc.dram_tensor("xt", [C, T], BF, kind="ExternalInput").ap()
    wqk = nc.dram_tensor("wqk", [C, 1024], BF, kind="ExternalInput").ap()
    wv = nc.dram_tensor("wv", [C, 256], BF, kind="ExternalInput").ap()
    wp = nc.dram_tensor("wp", [256, 1024], BF, kind="ExternalInput").ap()
    cos2 = nc.dram_tensor("cos2", [P, T], F32, kind="ExternalInput").ap()
    ss2 = nc.dram_tensor("ss2", [P, T], F32, kind="ExternalInput").ap()
    sc_exp = nc.dram_tensor("sc_exp", [P, 1], F32, kind="ExternalInput").ap()
    sc_out = nc.dram_tensor("sc_out", [P, 1], F32, kind="ExternalInput").ap()
    outT = nc.dram_tensor("outT", [C, T], F32, kind="ExternalOutput").ap()

    with tile.TileContext(nc) as tc:
        with (
            tc.tile_pool(name="consts", bufs=1) as consts,
            tc.tile_pool(name="tmps", bufs=6) as tmps,
            tc.tile_pool(name="epool", bufs=6) as epool,
            tc.tile_pool(name="opool", bufs=4) as opool,
            tc.tile_pool(name="ps_sc", bufs=2, space="PSUM") as ps_sc,
            tc.tile_pool(name="ps_y", bufs=4, space="PSUM") as ps_y,
        ):
            # ---- persistent SBUF loads ----
            xt_sb = consts.tile([P, NCC, T], BF)
            xt_v = xt.rearrange("(n p) t -> n p t", p=P)
            for i in range(NCC):
                nc.sync.dma_start(out=xt_sb[:, i, :], in_=xt_v[i])
            wqk_sb = consts.tile([P, NCC, 1024], BF)
            wqk_v = wqk.rearrange("(n p) m -> n p m", p=P)
            for i in range(NCC):
                nc.sync.dma_start(out=wqk_sb[:, i, :], in_=wqk_v[i])
            wv_sb = consts.tile([P, NCC, 256], BF)
            wv_v = wv.rearrange("(n p) m -> n p m", p=P)
            for i in range(NCC):
                nc.sync.dma_start(out=wv_sb[:, i, :], in_=wv_v[i])
            wp_sb = consts.tile([P, 2, 1024], BF)
            wp_v = wp.rearrange("(n p) m -> n p m", p=P)
            for i in range(2):
                nc.sync.dma_start(out=wp_sb[:, i, :], in_=wp_v[i])
            cos_sb = consts.tile([P, T], F32)
            nc.sync.dma_start(out=cos_sb, in_=cos2[:])
            ss_sb = consts.tile([P, T], F32)
            nc.sync.dma_start(out=ss_sb, in_=ss2[:])
            sce_sb = consts.tile([P, 1], F32)
            nc.sync.dma_start(out=sce_sb, in_=sc_exp[:])
            sco_sb = consts.tile([P, 1], F32)
            nc.sync.dma_start(out=sco_sb, in_=sc_out[:])

            # ---- V: token-major [k_token, vch] with interleaved ones ----
            # per 256-wide head pair: [v_even(64) | ones(128) | v_odd(64)]
            v_sb = consts.tile([P, NKC, 2, 256], BF)
            nc.vector.memset(v_sb[:, :, :, 64:192], 1.0)
            for tt in range(NKC):
                vp = ps_y.tile([P, 256], F32, tag="y")
                for kc in range(NCC):
                    nc.tensor.matmul(
                        vp,
                        lhsT=xt_sb[:, kc, tt * P:(tt + 1) * P],
                        rhs=wv_sb[:, kc, :],
                        start=(kc == 0),
                        stop=(kc == NCC - 1),
                    )
                vp4 = vp.rearrange("p (h2 two d) -> p h2 two d", two=2, d=64)
                nc.vector.tensor_copy(v_sb[:, tt, :, 0:64], vp4[:, :, 0, :])
                nc.vector.tensor_copy(v_sb[:, tt, :, 192:256], vp4[:, :, 1, :])

            # ---- Q/K channel-major + RoPE ----
            # wqk col blocks: QA[0:256) QB[256:512) KA[512:768) KB[768:1024)
            qk_sb = consts.tile([P, 4, T], BF)  # blk: q01, q23, k01, k23
            for blk in range(4):
                base_a = [0, 128, 512, 640][blk]
                base_b = base_a + 256
                for qt in range(NQT):
                    qs = slice(qt * QT, (qt + 1) * QT)
                    ps = ps_sc.tile([P, 1024], F32, tag="sc")
                    for kc in range(NCC):
                        nc.tensor.matmul(
                            ps[:, 0:QT],
                            lhsT=wqk_sb[:, kc, base_a:base_a + P],
                            rhs=xt_sb[:, kc, qs],
                            start=(kc == 0),
                            stop=(kc == NCC - 1),
                        )
                    for kc in range(NCC):
                        nc.tensor.matmul(
                            ps[:, QT:1024],
                            lhsT=wqk_sb[:, kc, base_b:base_b + P],
                            rhs=xt_sb[:, kc, qs],
                            start=(kc == 0),
                            stop=(kc == NCC - 1),
                        )
                    t1 = tmps.tile([P, QT], F32, tag="t1")
                    nc.vector.tensor_mul(t1, ps[:, 0:QT], cos_sb[:, qs])
                    t2 = tmps.tile([P, QT], F32, tag="t2")
                    nc.vector.tensor_mul(t2, ps[:, QT:1024], ss_sb[:, qs])
                    nc.gpsimd.tensor_add(qk_sb[:, blk, qs], t1, t2)

            # ---- causal attention, 2 heads per group packed ----
            y_sb = consts.tile([P, 2, T], BF)
            for grp in range(2):
                q_t = qk_sb[:, grp, :]
                k_t = qk_sb[:, 2 + grp, :]
                for qt in range(NQT):
                    qs = slice(qt * QT, (qt + 1) * QT)
                    KC = 4 * (qt + 1)  # causal k chunks
                    yA = ps_y.tile([P, QT], F32, tag="y")
                    yB = ps_y.tile([P, QT], F32, tag="y")
                    for kc in range(KC):
                        ks = slice(kc * P, (kc + 1) * P)
                        ps = ps_sc.tile([P, 1024], F32, tag="sc")
                        nc.tensor.matmul(ps[:, 0:QT], lhsT=k_t[0:64, ks],
                                         rhs=q_t[0:64, qs],
                                         start=True, stop=True)
                        nc.tensor.matmul(ps[:, QT:1024], lhsT=k_t[64:128, ks],
                                         rhs=q_t[64:128, qs],
                                         start=True, stop=True)
                        e = epool.tile([P, 1024], BF, tag="e")
                        nc.scalar.activation(e, ps, AF.Exp,
                                             scale=sce_sb[:, 0:1])
                        delta = kc * P - qt * QT
                        if delta >= 0:
                            # diagonal tile: keep where f - p - delta >= 0
                            e2 = e.rearrange("p (j f) -> p j f", j=2)
                            nc.gpsimd.affine_select(
                                e2, e2,
                                pattern=[[0, 2], [1, QT]],
                                compare_op=mybir.AluOpType.is_ge,
                                fill=0.0,
                                base=-delta,
                                channel_multiplier=-1,
                            )
                        nc.tensor.matmul(yA, lhsT=v_sb[:, kc, grp, 0:128],
                                         rhs=e[:, 0:QT],
                                         start=(kc == 0), stop=(kc == KC - 1))
                        nc.tensor.matmul(yB, lhsT=v_sb[:, kc, grp, 128:256],
                                         rhs=e[:, QT:1024],
                                         start=(kc == 0), stop=(kc == KC - 1))
                    # head A (even): y rows 0:64, denom rows 64:128
                    rcA = tmps.tile([P, QT], F32, tag="rc")
                    nc.vector.reciprocal(rcA[0:64, :], yA[64:128, :])
                    nc.vector.tensor_mul(y_sb[0:64, grp, qs], yA[0:64, :],
                                         rcA[0:64, :])
                    # head B (odd): denom rows 0:64, y rows 64:128
                    rcB = tmps.tile([P, QT], F32, tag="rc")
                    nc.vector.reciprocal(rcB[64:128, :], yB[0:64, :])
                    nc.vector.tensor_mul(y_sb[64:128, grp, qs],
                                         yB[64:128, :], rcB[64:128, :])

            # ---- projection: out.T[oc, t] partial ----
            for mt in range(8):
                ms = slice(mt * P, (mt + 1) * P)
                for qt in range(NQT):
                    qs = slice(qt * QT, (qt + 1) * QT)
                    pp = ps_y.tile([P, QT], F32, tag="y")
                    for ch in range(2):
                        nc.tensor.matmul(pp, lhsT=wp_sb[:, ch, ms],
                                         rhs=y_sb[:, ch, qs],
                                         start=(ch == 0), stop=(ch == 1))
                    ot = opool.tile([P, QT], F32, tag="ot")
                    nc.scalar.activation(ot, pp, AF.Copy,
                                         scale=sco_sb[:, 0:1])
                    nc.sync.dma_start(out=outT[ms, qs], in_=ot)

    nc.finalize()
    return nc


def _prep_inputs(x, cos, sin, w_qkv, w_proj):
    sq, am_q = _ternarize_host(w_qkv)
    sp, am_p = _ternarize_host(w_proj)

    cos_t = np.ascontiguousarray(cos[0, 0].T).astype(np.float32)  # [D, T]
    sin_t = np.ascontiguousarray(sin[0, 0].T).astype(np.float32)
    sgn = np.where(np.arange(D)[:, None] < D // 2, np.float32(-1.0),
                   np.float32(1.0))
    ss_t = sin_t * sgn
    cos2 = np.concatenate([cos_t, cos_t], axis=0)          # [128, T]
    ss2 = np.concatenate([ss_t, ss_t], axis=0)
    sc_exp = np.full((P, 1), am_q * am_q / np.sqrt(np.float32(D)),
                     np.float32)
    sc_out = np.full((P, 1), am_q * am_p, np.float32)

    perm = (np.arange(D) + D // 2) % D
    in_maps = []
    for core in range(N_CORES):
        b, g = divmod(core, HEADS_PER_CORE)
        heads = [4 * g + h for h in range(4)]
        q_rows = np.concatenate([np.arange(h * D, (h + 1) * D) for h in heads])
        qp_rows = np.concatenate([h * D + perm for h in heads])
        k_rows = C + q_rows
        kp_rows = C + qp_rows
        v_rows = 2 * C + q_rows
        wqk_block = np.concatenate(
            [sq[q_rows], sq[qp_rows], sq[k_rows], sq[kp_rows]], axis=0)
        wqk_t = np.ascontiguousarray(wqk_block.T).astype(BF16)   # [C, 1024]
        wv_t = np.ascontiguousarray(sq[v_rows].T).astype(BF16)   # [C, 256]
        wp_t = np.ascontiguousarray(sp[:, q_rows].T).astype(BF16)  # [256, C]
        xt = np.ascontiguousarray(x[b].T).astype(BF16)           # [C, T]
        in_maps.append({
            "xt": xt, "wqk": wqk_t, "wv": wv_t, "wp": wp_t,
            "cos2": cos2, "ss2": ss2, "sc_exp": sc_exp, "sc_out": sc_out,
        })
    return in_maps


def kernel(x, cos, sin, w_qkv, w_proj):
    x = np.asarray(x, dtype=np.float32)
    cos = np.asarray(cos, dtype=np.float32)
    sin = np.asarray(sin, dtype=np.float32)
    w_qkv = np.asarray(w_qkv, dtype=np.float32)
    w_proj = np.asarray(w_proj, dtype=np.float32)

    _install_ntff_hook()
    from concourse.bass_utils import run_bass_kernel_spmd

    if "nc" not in _CACHE:
        _CACHE["nc"] = _build_program()
    nc = _CACHE["nc"]

    in_maps = _prep_inputs(x, cos, sin, w_qkv, w_proj)
    trace = bool(os.environ.get("KERNEL_TRACE"))
    res = run_bass_kernel_spmd(nc, in_maps, core_ids=list(range(N_CORES)),
                               trace=trace)
    _CACHE["exec_time_ns"] = res.exec_time_ns

    out = np.zeros((B, T, C), dtype=np.float32)
    for core in range(N_CORES):
        b = core // HEADS_PER_CORE
        out[b] += res.results[core]["outT"].T
    return out
